# revision 5
# baseline (speedup 1.0000x reference)
"""ExtractSearchWindows Trainium2 kernel (8 NeuronCores, Bass/Tile).

out[b, h, w, dy*cv+dx, ky*8+kx] = uint8(P[b, h+off+dy+ky, w+off+dx+kx])
with P = zero-pad(inputs[:, 0], 7) and off = 3 - search_range.

The output (196.6 MB u8) is a byte-replication of a tiny input; the kernel
is bound by the SBUF->HBM DMA fabric (~435 GB/s/core = 16 SDMA engines x
27 GiB/s).  Work is sharded over (b, h): each of 8 cores emits 48 rows.

v2 layout: the host sends, per core, a compact "R4" array of 4 byte-shifted
(phi = w mod 4) sub-row copies so every device-side move is a 4-byte-aligned
u32 op: R4[seg, v, phi, jw] = P-bytes[base(seg) + v*PW + phi + 4*jw]
(12 v-rows x 4 phases x 48 B = 2304 B/seg vs 4224 B for the fully unfolded
form).  On device, the ACT engine expands R4 -> S (the nu=8 unfolded form:
sub-row u at word offset u//4 of phase u%4), then DVE builds output chunks
with strided u32 copies and HWDGE streams them out.  The first chunk is
emitted pixel-by-pixel so the out-DMA engines spin up ~4 us earlier.
"""
import os
import numpy as np

K = 8
MAX_SR = 3
B, H, W = 2, 192, 320
TP = MAX_SR + K // 2          # 7 pad per side
PW = W + 2 * TP               # 334
NCORES = 8
ROWS_PER_CORE = (B * H) // NCORES   # 48
WSEG = 40
NWSEG = W // WSEG             # 8
NSEG = ROWS_PER_CORE * NWSEG  # 384
NTILE = NSEG // 128           # 3
NCH = 2                       # w-chunks per segment
WCH = WSEG // NCH             # 20 pixels per chunk
NA = WCH // 4                 # 5

NV = 12                       # source rows per output row (cv-1+K for sr=2)
NJW = 12                      # u32 words per (v, phi) sub-row in R4
R4B = NV * 4 * NJW * 4        # 2304 bytes per seg
R4W = R4B // 4                # 576 words per seg
NJ = 44                       # bytes per unfolded sub-row in S
SEGB = NV * K * NJ            # 4224 bytes per seg in S
SW = SEGB // 4                # 1056 words

_PROG_CACHE = {}

FILL_PIXELS = int(os.environ.get("ESW_FILL_PIXELS", "6"))
# ACT's activation datapath mangles integer data (float path) — keep on DVE.
EXPAND_ACT = os.environ.get("ESW_EXPAND_ACT", "0") == "1"


def _make_r4_host(x, sr):
    """x: (B,1,H,W) f32 -> per-core [128, NTILE*R4B] u8 arrays."""
    off = MAX_SR - sr
    P = np.pad(x[:, 0], ((0, 0), (TP, TP), (TP, TP))).astype(np.uint8)
    cores = []
    for c in range(NCORES):
        b = (c * ROWS_PER_CORE) // H
        h0 = (c * ROWS_PER_CORE) % H
        flat = np.ascontiguousarray(P[b]).reshape(-1)
        base = (h0 + off) * PW + off
        s6 = np.lib.stride_tricks.as_strided(
            flat[base:], shape=(ROWS_PER_CORE, NWSEG, NV, 4, NJW, 4),
            strides=(PW, WSEG, PW, 1, 4, 1))
        arr = np.ascontiguousarray(s6).reshape(NSEG, R4B)
        cores.append(np.concatenate(
            [arr[t * 128:(t + 1) * 128] for t in range(NTILE)], axis=1))
    return cores


def _fill_units():
    """Chunk-0 emission plan: list of (w0, w1) pixel ranges."""
    units = [(w, w + 1) for w in range(FILL_PIXELS)]
    rest = WCH - FILL_PIXELS
    units.append((FILL_PIXELS, FILL_PIXELS + rest // 2))
    units.append((FILL_PIXELS + rest // 2, WCH))
    return units


def _build_program(sr):
    import concourse.bass as bass
    import concourse.bacc as bacc
    import concourse.mybir as mybir
    from concourse import tile

    cv = 2 * sr + 1
    out_seg_b = WSEG * cv * cv * K * K     # 64000
    ch_b = out_seg_b // NCH                # 32000
    pix_w = cv * cv * K * K // 4           # 400 u32 per pixel
    d_w = cv * K * K // 4                  # 80 u32 per dy block

    u8 = mybir.dt.uint8
    u32 = mybir.dt.uint32
    i32 = mybir.dt.int32
    nc = bacc.Bacc("TRN2", debug=False)
    r4_in = nc.declare_dram_parameter("r4_in", [128, NTILE * R4B], u8,
                                      isOutput=False)
    out = nc.declare_dram_parameter("out", [NSEG * out_seg_b], u8,
                                    isOutput=True)

    with tile.TileContext(nc) as tc:
        with tc.tile_pool(name="spool", bufs=1) as sp, \
             tc.tile_pool(name="tpool", bufs=3) as tp:
            R4 = sp.tile([128, NTILE * R4B], u8)
            S = sp.tile([128, NTILE * SEGB], u8)
            nc.sync.dma_start(R4[:, 0:R4B], r4_in[0:128, 0:R4B])
            nc.sync.dma_start(R4[:, R4B:], r4_in[0:128, R4B:])
            r32 = R4[:].bitcast(u32)
            s32 = S[:].bitcast(u32)
            r_row = NTILE * R4W            # u32 per partition in R4
            s_row = NTILE * SW             # u32 per partition in S

            def expand(t):
                # S[v, u, j] = R4[v, u%4, u//4 + j]  (u32 words)
                for half in range(2):      # u in [0,4) then [4,8)
                    dst = bass.AP(s32.tensor, t * SW + half * 4 * (NJ // 4),
                                  [[s_row, 128], [K * NJ // 4, NV],
                                   [NJ // 4, 4], [1, NJ // 4]])
                    src = bass.AP(r32.tensor, t * R4W + half,
                                  [[r_row, 128], [4 * NJW, NV],
                                   [NJW, 4], [1, NJ // 4]])
                    if EXPAND_ACT:
                        nc.scalar.copy(dst.bitcast(i32), src.bitcast(i32))
                    else:
                        nc.vector.tensor_copy(dst, src)

            def emit(t, ch, w0, w1, T, t32):
                """Copy pixels [w0,w1) of chunk (t,ch) into T, DMA them out."""
                for dy in range(cv):
                    for phi in range(4):
                        a_lo = -(-(w0 - phi) // 4)      # ceil
                        a_hi = (w1 - 1 - phi) // 4
                        if a_hi < a_lo:
                            continue
                        an = a_hi - a_lo + 1
                        src = bass.AP(
                            s32.tensor,
                            t * SW + dy * (K * NJ // 4) + phi * (NJ // 4)
                            + NA * ch + a_lo,
                            [[s_row, 128], [K * NJ // 4, K], [1, an],
                             [NJ // 4, cv], [1, 2]])
                        dst = bass.AP(
                            t32.tensor,
                            (4 * a_lo + phi) * pix_w + dy * d_w,
                            [[ch_b // 4, 128], [2, K], [4 * pix_w, an],
                             [K * K // 4, cv], [1, 2]])
                        nc.vector.tensor_copy(dst, src)
                gb0 = w0 * cv * cv * K * K
                gbn = (w1 - w0) * cv * cv * K * K
                dst_hbm = bass.AP(
                    out.ap().tensor,
                    (t * 128) * out_seg_b + ch * ch_b + gb0,
                    [[out_seg_b, 128], [1, gbn]])
                nc.sync.dma_start(dst_hbm, T[0:128, gb0:gb0 + gbn])

            expand(0)
            first = True
            for t in range(NTILE):
                if t > 0:
                    expand(t)
                for ch in range(NCH):
                    T = tp.tile([128, ch_b], u8)
                    t32 = T[:].bitcast(u32)
                    units = _fill_units() if first else [(0, WCH)]
                    first = False
                    for (w0, w1) in units:
                        emit(t, ch, w0, w1, T, t32)
    nc.compile()
    return nc


def _numpy_fallback(x, sr):
    cv = 2 * sr + 1
    off = MAX_SR - sr
    P = np.pad(x[:, 0], ((0, 0), (TP, TP), (TP, TP))).astype(np.uint8)
    out = np.empty((B, H, W, cv * cv, K * K), np.uint8)
    for dy in range(cv):
        for dx in range(cv):
            for ky in range(K):
                for kx in range(K):
                    out[:, :, :, dy * cv + dx, ky * K + kx] = \
                        P[:, off + dy + ky:off + dy + ky + H,
                          off + dx + kx:off + dx + kx + W]
    return out


def kernel(inputs, search_range):
    from concourse.bass_utils import run_bass_kernel_spmd

    x = np.asarray(inputs, dtype=np.float32)
    sr = int(np.asarray(search_range))
    if sr != 2 or x.shape != (B, 1, H, W):
        return _numpy_fallback(x, sr)

    cv = 2 * sr + 1
    if sr not in _PROG_CACHE:
        _PROG_CACHE[sr] = _build_program(sr)
    nc = _PROG_CACHE[sr]

    r4_cores = _make_r4_host(x, sr)
    res = run_bass_kernel_spmd(
        nc, [{"r4_in": r} for r in r4_cores], list(range(NCORES)))
    outs = [np.asarray(res.results[c]["out"]) for c in range(NCORES)]
    return np.concatenate(outs).reshape(B, H, W, cv * cv, K * K)


# revision 7
# speedup vs baseline: 1.1974x; 1.1974x over previous
"""ExtractSearchWindows Trainium2 kernel (8 NeuronCores, Bass/Tile).

out[b, h, w, dy*cv+dx, ky*8+kx] = uint8(P[b, h+off+dy+ky, w+off+dx+kx])
with P = zero-pad(inputs[:, 0], 7) and off = 3 - search_range.

Strategy: the output (196.6 MB u8) is a pure byte-replication of a tiny
input, so the kernel is bound by the SBUF->HBM DMA fabric (~435 GB/s/core
= 16 SDMA engines x ~27 GiB/s).  Work is sharded over (b, h): each of the
8 cores produces 48 output rows.

Host prep (tiny): pad+cast the 0.5 MB input to u8 and lay out, per core,
a 1.6 MB array of byte-shifted sub-rows "S" such that every device-side
expansion copy becomes a 4-byte-aligned strided uint32 tensor_copy
(phase-decomposed over w mod 4).  Device per core: 3 tiles x 128
segments (segment = 40-pixel row chunk); per tile, strided u32 DVE
copies -> 4 MB contiguous DMA-out in final (w, d, t) byte order.
Pipeline: chunk 0 is emitted in three pixel-range groups so the first
out-DMA launches after ~1/3 of the chunk's copies, and chunk 1 in two
half-chunks so the out-DMA queue never runs dry while DVE builds it.
"""
import numpy as np

K = 8
MAX_SR = 3
B, H, W = 2, 192, 320
TP = MAX_SR + K // 2          # 7 pad per side
PW = W + 2 * TP               # 334
NCORES = 8
ROWS_PER_CORE = (B * H) // NCORES   # 48
WSEG = 40
NWSEG = W // WSEG             # 8
NSEG = ROWS_PER_CORE * NWSEG  # 384
NTILE = NSEG // 128           # 3
NCH = 2                       # w-chunks per segment
WCH = WSEG // NCH             # 20 pixels per chunk
NA = WCH // 4                 # 5

_PROG_CACHE = {}


def _geom(sr):
    cv = 2 * sr + 1
    off = MAX_SR - sr
    nv = cv - 1 + K                  # source rows per output row
    nu = 4 + cv - 1                  # shifted sub-rows: phi + dx
    nj = 4 * (WSEG // 4 - 1) + (K - 1) + 1  # sub-row bytes (covers all chunks)
    nj = (nj + 3) // 4 * 4                  # pad to mult of 4 -> 44
    return cv, off, nv, nu, nj


def _make_s_host(x, sr):
    """x: (B,1,H,W) f32 -> per-core list of [NSEG, nv*nu*nj] u8 arrays."""
    cv, off, nv, nu, nj = _geom(sr)
    P = np.pad(x[:, 0], ((0, 0), (TP, TP), (TP, TP))).astype(np.uint8)
    cores = []
    for c in range(NCORES):
        b = (c * ROWS_PER_CORE) // H
        h0 = (c * ROWS_PER_CORE) % H
        flat = np.ascontiguousarray(P[b]).reshape(-1)
        base = (h0 + off) * PW + off
        s5 = np.lib.stride_tricks.as_strided(
            flat[base:], shape=(ROWS_PER_CORE, NWSEG, nv, nu, nj),
            strides=(PW, WSEG, PW, 1, 1))
        cores.append(np.ascontiguousarray(s5).reshape(NSEG, nv * nu * nj))
    return cores


def _strip_const_memsets(nc):
    """Drop the unused const-AP Memset preamble (saves ~0.4 us of startup)."""
    import concourse.mybir as mybir
    entry = nc.main_func.blocks[0]
    keep = []
    for inst in entry.instructions:
        if isinstance(inst, mybir.InstMemset) and inst.outs and \
                str(inst.outs[0].memsetref).startswith("const-"):
            continue
        keep.append(inst)
    entry.instructions[:] = keep


def _build_program(sr):
    import concourse.bass as bass
    import concourse.bacc as bacc
    import concourse.mybir as mybir
    from concourse import tile

    cv, off, nv, nu, nj = _geom(sr)
    segb = nv * nu * nj
    segw = segb // 4
    out_seg_b = WSEG * cv * cv * K * K
    ch_b = out_seg_b // NCH
    ch_w = ch_b // 4
    d_i32 = cv * K * K // 4        # u32 per pixel per dy (= 80 for cv=5)
    pix_i32 = cv * cv * K * K // 4  # u32 per pixel (= 400 for cv=5)

    u8 = mybir.dt.uint8
    u32 = mybir.dt.uint32
    nc = bacc.Bacc("TRN2", debug=False)
    _strip_const_memsets(nc)
    s_in = nc.declare_dram_parameter("s_in", [NSEG, segb], u8, isOutput=False)
    out = nc.declare_dram_parameter("out", [NSEG * out_seg_b], u8, isOutput=True)

    with tile.TileContext(nc) as tc:
        with tc.tile_pool(name="spool", bufs=1) as sp, \
             tc.tile_pool(name="tpool", bufs=3) as tp:
            # All of S stays resident (12.7 KB/partition).  Two DMAs: tile 0
            # first so compute starts early, then tiles 1..NTILE-1.
            S = sp.tile([128, NTILE * segb], u8)
            nc.sync.dma_start(S[:, 0:segb], s_in[0:128, :])
            rest_src = bass.AP(s_in.ap().tensor, 128 * segb,
                               [[segb, 128], [128 * segb, NTILE - 1],
                                [1, segb]])
            rest_dst = bass.AP(S[:].tensor, segb,
                               [[NTILE * segb, 128], [segb, NTILE - 1],
                                [1, segb]])
            nc.sync.dma_start(rest_dst, rest_src)
            s32 = S[:].bitcast(u32)

            def emit(t, ch, w0, w1, T, t32):
                """Copy pixels [w0,w1) of chunk (t,ch) into T; DMA them out."""
                for dy in range(cv):
                    for phi in range(4):
                        a_lo = -(-(w0 - phi) // 4)      # ceil
                        a_hi = (w1 - 1 - phi) // 4
                        if a_hi < a_lo:
                            continue
                        an = a_hi - a_lo + 1
                        src = bass.AP(
                            s32.tensor,
                            t * segw + dy * (nu * nj // 4)
                            + phi * (nj // 4) + NA * ch + a_lo,
                            [[NTILE * segw, 128],
                             [nu * nj // 4, K],  # ky: next src row
                             [1, an],            # a: +4 bytes
                             [nj // 4, cv],      # dx: next sub-row
                             [1, 2]])            # kx pair
                        dst = bass.AP(
                            t32.tensor,
                            (4 * a_lo + phi) * pix_i32 + dy * d_i32,
                            [[ch_w, 128],
                             [2, K],             # ky: +8 bytes
                             [4 * pix_i32, an],  # a: +4 pixels
                             [K * K // 4, cv],   # dx: +64 bytes
                             [1, 2]])            # kx pair
                        nc.vector.tensor_copy(dst, src)
                gb0 = w0 * cv * cv * K * K
                gbn = (w1 - w0) * cv * cv * K * K
                dst_hbm = bass.AP(
                    out.ap().tensor,
                    (t * 128) * out_seg_b + ch * ch_b + gb0,
                    [[out_seg_b, 128], [1, gbn]])
                nc.sync.dma_start(dst_hbm, T[0:128, gb0:gb0 + gbn])

            # Pixel-range units per chunk index (t*NCH+ch): chunk 0 in three
            # groups (pipeline fill), chunk 1 in two halves (keeps the DMA
            # queue fed while DVE builds it), the rest whole.
            unit_plan = {0: [(0, 8), (8, 16), (16, 20)],
                         1: [(0, 10), (10, 20)]}
            for t in range(NTILE):
                for ch in range(NCH):
                    T = tp.tile([128, ch_b], u8)
                    t32 = T[:].bitcast(u32)
                    units = unit_plan.get(t * NCH + ch, [(0, WCH)])
                    for (w0, w1) in units:
                        emit(t, ch, w0, w1, T, t32)
    nc.compile()
    return nc


def _numpy_fallback(x, sr):
    cv, off, _, _, _ = _geom(sr)
    P = np.pad(x[:, 0], ((0, 0), (TP, TP), (TP, TP))).astype(np.uint8)
    out = np.empty((B, H, W, cv * cv, K * K), np.uint8)
    for dy in range(cv):
        for dx in range(cv):
            for ky in range(K):
                for kx in range(K):
                    out[:, :, :, dy * cv + dx, ky * K + kx] = \
                        P[:, off + dy + ky:off + dy + ky + H,
                          off + dx + kx:off + dx + kx + W]
    return out


def kernel(inputs, search_range):
    from concourse.bass_utils import run_bass_kernel_spmd

    x = np.asarray(inputs, dtype=np.float32)
    sr = int(np.asarray(search_range))
    if sr != 2 or x.shape != (B, 1, H, W):
        return _numpy_fallback(x, sr)

    cv = 2 * sr + 1
    if sr not in _PROG_CACHE:
        _PROG_CACHE[sr] = _build_program(sr)
    nc = _PROG_CACHE[sr]

    s_cores = _make_s_host(x, sr)
    res = run_bass_kernel_spmd(
        nc, [{"s_in": s} for s in s_cores], list(range(NCORES)))
    outs = [np.asarray(res.results[c]["out"]) for c in range(NCORES)]
    return np.concatenate(outs).reshape(B, H, W, cv * cv, K * K)
